# revision 16
# baseline (speedup 1.0000x reference)
"""DockingScorePredictor Trainium2 kernel — compacted-pair MLP.

Data-parallel over complexes: 8 cores, one complex (512 protein x 64 ligand
atoms) per core.  The host drops masked pairs (dist >= cutoff, ~55%) and
packs the survivors into tiles of 512 pairs.  Because atom-type vocabularies
are tiny (20/16), the whole first layer collapses into ONE K=68 matmul per
tile: rhs rows = [32 radial-basis | 20 protein-type one-hot | 16 ligand-type
one-hot], lhsT rows = [W1c | prot_emb@W1a | lig_emb@W1b]; b1 rides the relu
bias port.  Per tile the MLP is 3 matmuls (z1, W2, W3) plus a K=128 matmul
((b3/128 bcast)^T @ ones) that injects b3*ones into z3's PSUM — only ~31
tiles run instead of 64.  K=1 injects are NOT used: the PE_HAM activity
monitor treats them as idle and clock-gates the array to 1.2 GHz (measured:
mixing K=1 drops back-to-back matmuls from 216 ns to 539 ns; K>=68 holds
2.4 GHz).

Engine layout per tile: relu2 -> ACT (bias port), relu3+pair-sum -> DVE
(op0=max, op1=add + accum_out — DVE's accumulator requires op1=add, hence
the b3 PSUM inject; DVE's accumulator readout is ~9 ns vs ACT's ~181 ns),
relu1 3/5 on ACT, 2/5 on DVE, balancing both at ~1 us/tile.  Engine reads
cannot cross PSUM bank boundaries, so all PSUM tiles are one bank
([128,512] f32).

DMA-dispatch latency (~0.65 us per dma_start, serialized per queue)
dominates startup, so: rhs tiles stream per-tile from tile-contiguous DRAM
via the otherwise-idle GpSimd queue, and the small constants are packed
into four wide tensors (one dispatch each) ordered by first use on the
Sync queue.

Padding columns (all-zero rhs) produce a constant h3 = relu(W3^T relu(W2^T
relu(b1) + b2) + b3) which the host pre-computes (with matching bf16
quantization) and subtracts from the pair-sum.  All MLP matmuls are bf16
(FWL keeps LDWEIGHTS off the critical path); rel tolerance is 2e-2 and
bf16 quantization lands ~3e-3.
"""
import numpy as np
import ml_dtypes
from contextlib import ExitStack

import concourse.bass as bass
import concourse.bacc as bacc
import concourse.tile as tile
from concourse import mybir
from concourse import bass_utils

F32 = mybir.dt.float32
BF16 = mybir.dt.bfloat16
AF = mybir.ActivationFunctionType
ALU = mybir.AluOpType

B, P, L = 8, 512, 64
H, RB = 128, 32
NPT, NLT = 20, 16
CUTOFF = 8.0
N_CORES = 8
WIDTH = 0.5 * CUTOFF / RB + 1e-8
K1 = RB + NPT + NLT + 1        # 68 feature rows + const-1 row (b1)
TCOLS = 512                    # pair columns per tile (one PSUM bank)

_CACHE = {}
BF = ml_dtypes.bfloat16


def _r1_on_act(ti):
    return (ti % 15) % 2 == 0    # 8/15 of relu1 ops on ACT


def _build_nc(nt):
    nc = bacc.Bacc("TRN2", target_bir_lowering=False, debug=False,
                   num_devices=N_CORES)
    d = {}

    def inp(name, shape, dt):
        d[name] = nc.dram_tensor(name, shape, dt, kind="ExternalInput").ap()

    inp("rhs_all", [nt * K1, TCOLS], BF16)     # tile-contiguous
    inp("lhs1", [K1, H], BF16)
    inp("wpack", [H, 3 * H], BF16)  # [W2 | W3 | b3/128 bcast]
    # vecs cols: b1,b2,br1,padc3,recb,Wr2; col6 row0=br2, col7 row0=gt0
    inp("vecs", [H, 8], F32)
    inp("Wr1", [H, H], F32)

    score_ap = nc.dram_tensor("score", [1, 1], F32, kind="ExternalOutput").ap()

    with tile.TileContext(nc) as tc:
        with ExitStack() as ctx:
            const = ctx.enter_context(tc.tile_pool(name="const", bufs=1))
            rhsp = ctx.enter_context(tc.tile_pool(name="rhsp", bufs=4))
            abuf = ctx.enter_context(tc.tile_pool(name="abuf", bufs=3))
            psZ1 = ctx.enter_context(tc.tile_pool(name="psZ1", bufs=3, space="PSUM"))
            psZ2 = ctx.enter_context(tc.tile_pool(name="psZ2", bufs=3, space="PSUM"))
            psZ3 = ctx.enter_context(tc.tile_pool(name="psZ3", bufs=2, space="PSUM"))

            rhs_t, z1_t, a1_t, z2_t, a2_t, z3_t = {}, {}, {}, {}, {}, {}

            def issue_rhs(ti):
                rhs = rhsp.tile([K1, TCOLS], BF16, tag="rhs", name=f"rhs{ti}")
                nc.gpsimd.dma_start(out=rhs[:, :],
                                    in_=d["rhs_all"][K1 * ti:K1 * (ti + 1), :])
                rhs_t[ti] = rhs

            # constants, ordered by first use; packed to keep the per-DMA
            # dispatch count down (~0.65us each, serialized per queue)
            t = {}
            t["lhs1"] = const.tile([K1, H], BF16, tag="lhs1", name="lhs1")
            nc.sync.dma_start(out=t["lhs1"], in_=d["lhs1"])
            for ti in range(min(2, nt)):
                issue_rhs(ti)
            t["vecs"] = const.tile([H, 8], F32, tag="vecs", name="vecs")
            nc.sync.dma_start(out=t["vecs"], in_=d["vecs"])
            t["wpack"] = const.tile([H, 3 * H], BF16, tag="wpack", name="wpack")
            nc.sync.dma_start(out=t["wpack"], in_=d["wpack"])

            b1, b2, br1 = (t["vecs"][:, i:i + 1] for i in range(3))
            padc3, recb, Wr2 = (t["vecs"][:, i:i + 1] for i in range(3, 6))
            br2 = t["vecs"][0:1, 6:7]
            gt0 = t["vecs"][0:1, 7:8]
            W2 = t["wpack"][:, 0:H]
            W3 = t["wpack"][:, H:2 * H]
            injW = t["wpack"][:, 2 * H:3 * H]

            # all-ones rhs for the b3 inject, built on-device
            oneb = const.tile([H, TCOLS], BF16, tag="oneb", name="oneb")
            nc.gpsimd.memset(oneb[:, :], 1.0)

            # prime the ACT function table (Relu) so the ~1.3us table load
            # overlaps the initial DMAs instead of stalling relu2(0)
            warm = const.tile([1, 1], F32, tag="warm", name="warm")
            nc.vector.memset(warm[:, :], 0.0)
            nc.scalar.activation(out=warm[:, :], in_=warm[:, :], func=AF.Relu,
                                 bias=0.0, scale=1.0)

            # Wr1 is only needed by the head; dispatch it last
            t["Wr1"] = const.tile([H, H], F32, tag="Wr1", name="Wr1")
            nc.sync.dma_start(out=t["Wr1"], in_=d["Wr1"])

            accD = const.tile([H, nt], F32, tag="accD", name="accD")

            # PE_HAM warm-up: ~8 x 512-col matmuls ramp the clock gate to
            # 2.4 GHz while the first rhs tiles are still in flight
            for w in range(8):
                zw = psZ1.tile([H, TCOLS], F32, tag="z1", name=f"zw{w}")
                nc.tensor.matmul(out=zw[:, :], lhsT=t["lhs1"][:, :],
                                 rhs=oneb[0:K1, :], start=True, stop=True)

            for step in range(nt + 8):
                # S6: relu3 + pair-sum accumulate (DVE; b3 already in PSUM)
                ti = step - 7
                if 0 <= ti < nt:
                    a3 = abuf.tile([H, TCOLS], BF16, tag="a3", name=f"a3_{ti}")
                    nc.vector.tensor_scalar(out=a3[:, :],
                                            in0=z3_t.pop(ti)[:, :],
                                            scalar1=0.0, scalar2=0.0,
                                            op0=ALU.max, op1=ALU.add,
                                            accum_out=accD[:, ti:ti + 1])
                # S5: layer-3 matmul + K=128 b3 inject
                ti = step - 6
                if 0 <= ti < nt:
                    z3 = psZ3.tile([H, TCOLS], F32, tag="z3", name=f"z3_{ti}")
                    nc.tensor.matmul(out=z3[:, :], lhsT=W3,
                                     rhs=a2_t.pop(ti)[:, :],
                                     start=True, stop=False)
                    nc.tensor.matmul(out=z3[:, :], lhsT=injW, rhs=oneb[:, :],
                                     start=False, stop=True)
                    z3_t[ti] = z3
                # S4: relu2 (ACT)
                ti = step - 5
                if 0 <= ti < nt:
                    a2 = abuf.tile([H, TCOLS], BF16, tag="a2", name=f"a2_{ti}")
                    nc.scalar.activation(out=a2[:, :], in_=z2_t.pop(ti)[:, :],
                                         func=AF.Relu, bias=b2, scale=1.0)
                    a2_t[ti] = a2
                # S3: layer-2 matmul
                ti = step - 4
                if 0 <= ti < nt:
                    z2 = psZ2.tile([H, TCOLS], F32, tag="z2", name=f"z2_{ti}")
                    nc.tensor.matmul(out=z2[:, :], lhsT=W2,
                                     rhs=a1_t.pop(ti)[:, :],
                                     start=True, stop=True)
                    z2_t[ti] = z2
                # S2: relu1 (b1 already in z1 via the const-1 row);
                # 8/15 ACT, 7/15 DVE to balance the engines
                ti = step - 3
                if 0 <= ti < nt:
                    a1 = abuf.tile([H, TCOLS], BF16, tag="a1", name=f"a1_{ti}")
                    z1 = z1_t.pop(ti)
                    if _r1_on_act(ti):
                        nc.scalar.activation(out=a1[:, :], in_=z1[:, :],
                                             func=AF.Relu, bias=0.0, scale=1.0)
                    else:
                        nc.vector.tensor_scalar(out=a1[:, :], in0=z1[:, :],
                                                scalar1=0.0, scalar2=0.0,
                                                op0=ALU.max, op1=ALU.add)
                    a1_t[ti] = a1
                # S1: fused layer-1 matmul (rb + type one-hots)
                ti = step - 2
                if 0 <= ti < nt:
                    z1 = psZ1.tile([H, TCOLS], F32, tag="z1", name=f"z1_{ti}")
                    nc.tensor.matmul(out=z1[:, :], lhsT=t["lhs1"][:, :],
                                     rhs=rhs_t.pop(ti)[:, :],
                                     start=True, stop=True)
                    z1_t[ti] = z1
                # S0: stream the next rhs tile (2 pre-issued; the gpsimd
                # queue blocks on pool WARs, throttling to just-in-time)
                ti = step + 2
                if 2 <= ti < nt:
                    issue_rhs(ti)

            # ---- head ----
            tot = const.tile([H, 1], F32, tag="tot", name="tot")
            nc.vector.tensor_reduce(out=tot[:, :], in_=accD[:, :],
                                    axis=mybir.AxisListType.X, op=ALU.add)
            # repr = (tot - padc3) * recb
            repr_ = const.tile([H, 1], F32, tag="repr", name="repr_")
            nc.vector.scalar_tensor_tensor(out=repr_[:, :], in0=tot[:, :],
                                           scalar=padc3, in1=recb,
                                           op0=ALU.subtract, op1=ALU.mult)
            r1_ps = psZ2.tile([H, 1], F32, tag="z2", name="r1_ps")
            nc.tensor.matmul(out=r1_ps[:, :], lhsT=t["Wr1"][:, :],
                             rhs=repr_[:, :], start=True, stop=True)
            r1 = const.tile([H, 1], F32, tag="r1", name="r1")
            nc.scalar.activation(out=r1[:, :], in_=r1_ps[:, :], func=AF.Relu,
                                 bias=br1, scale=1.0)
            sc_ps = psZ3.tile([1, 1], F32, tag="z3", name="sc_ps")
            nc.tensor.matmul(out=sc_ps[:, :], lhsT=Wr2,
                             rhs=r1[:, :], start=True, stop=True)
            scf = const.tile([1, 1], F32, tag="scf", name="scf")
            nc.vector.scalar_tensor_tensor(out=scf[:, :], in0=sc_ps[:, :],
                                           scalar=br2, in1=gt0,
                                           op0=ALU.add, op1=ALU.mult)
            nc.sync.dma_start(out=score_ap, in_=scf[:, :])

    nc.compile()
    return nc


def _get_nc(nt):
    key = ("nc", nt)
    if key not in _CACHE:
        _CACHE[key] = _build_nc(nt)
    return _CACHE[key]


def kernel(protein_pos, ligand_pos, prot_emb, lig_emb,
           W1, b1, W2, b2, W3, b3, Wr1, br1, Wr2, br2,
           protein_atom_type, ligand_atom_type, protein_batch, ligand_batch):
    protein_pos = np.asarray(protein_pos, dtype=np.float32).reshape(B, P, 3)
    ligand_pos = np.asarray(ligand_pos, dtype=np.float32).reshape(B, L, 3)
    prot_emb = np.asarray(prot_emb, dtype=np.float32)
    lig_emb = np.asarray(lig_emb, dtype=np.float32)
    W1 = np.asarray(W1, dtype=np.float32)
    W2 = np.asarray(W2, dtype=np.float32)
    W3 = np.asarray(W3, dtype=np.float32)
    b1 = np.asarray(b1, dtype=np.float32).reshape(H)
    b2 = np.asarray(b2, dtype=np.float32).reshape(H)
    b3 = np.asarray(b3, dtype=np.float32).reshape(H)
    ptype = np.asarray(protein_atom_type).astype(np.int64).reshape(B, P)
    ltype = np.asarray(ligand_atom_type).astype(np.int64).reshape(B, L)

    # fused layer-1 weights: [W1c | prot_emb@W1a | lig_emb@W1b]
    W1a, W1b, W1c = W1[0:H], W1[H:2 * H], W1[2 * H:2 * H + RB]
    lhs1 = np.zeros((K1, H), dtype=np.float32)
    lhs1[0:RB] = W1c
    lhs1[RB:RB + NPT] = prot_emb @ W1a
    lhs1[RB + NPT:K1 - 1] = lig_emb @ W1b
    lhs1[K1 - 1] = b1
    lhs1_bf = lhs1.astype(BF)
    W2_bf = W2.astype(BF)
    W3_bf = W3.astype(BF)
    # the inject matmul computes sum_k (b3[m]/128) * 1 = b3[m]; /128 is a
    # power of two so bf16(b3/128) loses nothing over bf16(b3)
    injW = np.tile((b3 / 128.0).astype(BF)[None, :], (H, 1))
    wpack = np.concatenate([W2_bf, W3_bf, injW], axis=1)

    # padding column output, with device-matching bf16 quantization
    b1q = b1.astype(BF).astype(np.float32)
    a1p = np.maximum(b1q, 0.0).astype(BF).astype(np.float32)
    a2p = np.maximum(W2_bf.astype(np.float32).T @ a1p + b2, 0.0)
    a2p = a2p.astype(BF).astype(np.float32)
    b3q = (b3 / 128.0).astype(BF).astype(np.float32) * 128.0
    c3 = np.maximum(W3_bf.astype(np.float32).T @ a2p + b3q, 0.0)

    centers = np.linspace(0.0, CUTOFF, RB, dtype=np.float32)

    # per-complex compaction
    rb_l, pt_l, lt_l, nv_l = [], [], [], []
    for b in range(B):
        diff = protein_pos[b][:, None, :] - ligand_pos[b][None, :, :]
        dist = np.sqrt((diff * diff).sum(-1, dtype=np.float32))
        pidx, lidx = np.nonzero(dist < np.float32(CUTOFF))
        dv = dist[pidx, lidx]
        rb_l.append(np.exp(-0.5 * ((dv[:, None] - centers) / WIDTH) ** 2,
                           dtype=np.float32))
        pt_l.append(ptype[b][pidx])
        lt_l.append(ltype[b][lidx])
        nv_l.append(len(dv))

    nt = max(2, -(-max(nv_l) // TCOLS))
    cols = nt * TCOLS

    in_maps = []
    for b in range(B):
        nv = nv_l[b]
        rhs = np.zeros((K1, cols), dtype=BF)
        rhs[0:RB, :nv] = rb_l[b].T.astype(BF)
        ar = np.arange(nv)
        onehot = np.zeros((NPT + NLT, nv), dtype=BF)
        onehot[pt_l[b], ar] = 1.0
        onehot[NPT + lt_l[b], ar] = 1.0
        rhs[RB:K1 - 1, :nv] = onehot
        rhs[K1 - 1, :] = 1.0          # const-1 row: adds b1 everywhere
        # tile-contiguous layout: [nt*K1, TCOLS]
        rhs_t = np.ascontiguousarray(
            rhs.reshape(K1, nt, TCOLS).transpose(1, 0, 2)).reshape(
                nt * K1, TCOLS)
        vecs = np.zeros((H, 8), dtype=np.float32)
        vecs[:, 0] = b1
        vecs[:, 1] = b2
        vecs[:, 2] = np.asarray(br1, np.float32).reshape(H)
        vecs[:, 3] = (cols - nv) * c3
        vecs[:, 4] = 1.0 / max(nv, 1.0)
        vecs[:, 5] = np.asarray(Wr2, np.float32).reshape(H)
        vecs[0, 6] = np.float32(np.asarray(br2).reshape(()))
        vecs[0, 7] = 1.0 if nv > 0 else 0.0
        m = {
            "rhs_all": rhs_t,
            "lhs1": lhs1_bf,
            "wpack": wpack,
            "vecs": vecs,
            "Wr1": np.asarray(Wr1, np.float32),
        }
        in_maps.append(m)

    nc = _get_nc(nt)
    res = bass_utils.run_bass_kernel_spmd(nc, in_maps,
                                          core_ids=list(range(N_CORES)))
    out = np.array([res.results[b]["score"][0, 0] for b in range(B)],
                   dtype=np.float32)
    return out


# revision 20
# speedup vs baseline: 1.1712x; 1.1712x over previous
"""DockingScorePredictor Trainium2 kernel — compacted-pair MLP.

Data-parallel over complexes: 8 cores, one complex (512 protein x 64 ligand
atoms) per core.  The host drops masked pairs (dist >= cutoff, ~55%) and
packs the survivors into tiles of 512 pairs.  Because atom-type vocabularies
are tiny (20/16), the whole first layer collapses into ONE K=68 matmul per
tile: rhs rows = [32 radial-basis | 20 protein-type one-hot | 16 ligand-type
one-hot], lhsT rows = [W1c | prot_emb@W1a | lig_emb@W1b]; b1 rides the relu
bias port.  Per tile the MLP is 3 matmuls (z1, W2, W3) plus a K=128 matmul
((b3/128 bcast)^T @ ones) that injects b3*ones into z3's PSUM — only ~31
tiles run instead of 64.  K=1 injects are NOT used: the PE_HAM activity
monitor treats them as idle and clock-gates the array to 1.2 GHz (measured:
mixing K=1 drops back-to-back matmuls from 216 ns to 539 ns; K>=68 holds
2.4 GHz).

Engine layout per tile: relu2 -> ACT (bias port), relu3+pair-sum -> DVE
(op0=max, op1=add + accum_out — DVE's accumulator requires op1=add, hence
the b3 PSUM inject; DVE's accumulator readout is ~9 ns vs ACT's ~181 ns),
relu1 3/5 on ACT, 2/5 on DVE, balancing both at ~1 us/tile.  Engine reads
cannot cross PSUM bank boundaries, so all PSUM tiles are one bank
([128,512] f32).

DMA-dispatch latency (~0.65 us per dma_start, serialized per queue)
dominates startup, so: rhs tiles stream per-tile from tile-contiguous DRAM
via the otherwise-idle GpSimd queue, and the small constants are packed
into four wide tensors (one dispatch each) ordered by first use on the
Sync queue.

Padding columns (all-zero rhs) produce a constant h3 = relu(W3^T relu(W2^T
relu(b1) + b2) + b3) which the host pre-computes (with matching bf16
quantization) and subtracts from the pair-sum.  All MLP matmuls are bf16
(FWL keeps LDWEIGHTS off the critical path); rel tolerance is 2e-2 and
bf16 quantization lands ~3e-3.
"""
import numpy as np
import ml_dtypes
from contextlib import ExitStack

import concourse.bass as bass
import concourse.bacc as bacc
import concourse.tile as tile
from concourse import mybir
from concourse import bass_utils

F32 = mybir.dt.float32
BF16 = mybir.dt.bfloat16
AF = mybir.ActivationFunctionType
ALU = mybir.AluOpType

B, P, L = 8, 512, 64
H, RB = 128, 32
NPT, NLT = 20, 16
CUTOFF = 8.0
N_CORES = 8
WIDTH = 0.5 * CUTOFF / RB + 1e-8
K1 = RB + NPT + NLT + 1        # 68 feature rows + const-1 row (b1)
TCOLS = 512                    # pair columns per tile (one PSUM bank)

_CACHE = {}
BF = ml_dtypes.bfloat16


def _r1_on_act(ti):
    return (ti % 15) % 2 == 0    # 8/15 of relu1 ops on ACT


def _build_nc(nt):
    nc = bacc.Bacc("TRN2", target_bir_lowering=False, debug=False,
                   num_devices=N_CORES)
    d = {}

    def inp(name, shape, dt):
        d[name] = nc.dram_tensor(name, shape, dt, kind="ExternalInput").ap()

    inp("rhs_all", [nt * K1, TCOLS], BF16)     # tile-contiguous
    inp("lhs1", [K1, H], BF16)
    inp("wpack", [H, 3 * H], BF16)  # [W2 | W3 | b3/128 bcast]
    # vecs cols: b1,b2,br1,padc3,recb,Wr2; col6 row0=br2, col7 row0=gt0
    inp("vecs", [H, 8], F32)
    inp("Wr1", [H, H], F32)

    score_ap = nc.dram_tensor("score", [1, 1], F32, kind="ExternalOutput").ap()

    with tile.TileContext(nc) as tc:
        with ExitStack() as ctx:
            const = ctx.enter_context(tc.tile_pool(name="const", bufs=1))
            rhsp = ctx.enter_context(tc.tile_pool(name="rhsp", bufs=6))
            abuf = ctx.enter_context(tc.tile_pool(name="abuf", bufs=3))
            psZ1 = ctx.enter_context(tc.tile_pool(name="psZ1", bufs=3, space="PSUM"))
            psZ2 = ctx.enter_context(tc.tile_pool(name="psZ2", bufs=3, space="PSUM"))
            psZ3 = ctx.enter_context(tc.tile_pool(name="psZ3", bufs=2, space="PSUM"))

            rhs_t, z1_t, a1_t, z2_t, a2_t, z3_t = {}, {}, {}, {}, {}, {}

            def issue_rhs(ti):
                rhs = rhsp.tile([K1, TCOLS], BF16, tag="rhs", name=f"rhs{ti}")
                nc.gpsimd.dma_start(out=rhs[:, :],
                                    in_=d["rhs_all"][K1 * ti:K1 * (ti + 1), :])
                rhs_t[ti] = rhs

            # constants, ordered by first use; packed to keep the per-DMA
            # dispatch count down (~0.65us each, serialized per queue)
            t = {}
            t["lhs1"] = const.tile([K1, H], BF16, tag="lhs1", name="lhs1")
            nc.sync.dma_start(out=t["lhs1"], in_=d["lhs1"])
            for ti in range(min(2, nt)):
                issue_rhs(ti)
            t["vecs"] = const.tile([H, 8], F32, tag="vecs", name="vecs")
            nc.sync.dma_start(out=t["vecs"], in_=d["vecs"])
            t["wpack"] = const.tile([H, 3 * H], BF16, tag="wpack", name="wpack")
            nc.sync.dma_start(out=t["wpack"], in_=d["wpack"])

            b1, b2, br1 = (t["vecs"][:, i:i + 1] for i in range(3))
            padc3, recb, Wr2 = (t["vecs"][:, i:i + 1] for i in range(3, 6))
            br2 = t["vecs"][0:1, 6:7]
            gt0 = t["vecs"][0:1, 7:8]
            W2 = t["wpack"][:, 0:H]
            W3 = t["wpack"][:, H:2 * H]
            injW = t["wpack"][:, 2 * H:3 * H]

            # all-ones rhs for the b3 inject, built on-device
            oneb = const.tile([H, TCOLS], BF16, tag="oneb", name="oneb")
            nc.gpsimd.memset(oneb[:, :], 1.0)

            # prime the ACT function table (Relu) so the ~1.3us table load
            # overlaps the initial DMAs instead of stalling relu2(0)
            warm = const.tile([1, 1], F32, tag="warm", name="warm")
            nc.vector.memset(warm[:, :], 0.0)
            nc.scalar.activation(out=warm[:, :], in_=warm[:, :], func=AF.Relu,
                                 bias=0.0, scale=1.0)

            # Wr1 is only needed by the head; dispatch it last
            t["Wr1"] = const.tile([H, H], F32, tag="Wr1", name="Wr1")
            nc.sync.dma_start(out=t["Wr1"], in_=d["Wr1"])

            accD = const.tile([H, nt], F32, tag="accD", name="accD")
            next_rhs = min(2, nt)

            for step in range(nt + 8):
                # S6: relu3 + pair-sum accumulate (DVE; b3 already in PSUM)
                ti = step - 7
                if 0 <= ti < nt:
                    a3 = abuf.tile([H, TCOLS], BF16, tag="a3", name=f"a3_{ti}")
                    nc.vector.tensor_scalar(out=a3[:, :],
                                            in0=z3_t.pop(ti)[:, :],
                                            scalar1=0.0, scalar2=0.0,
                                            op0=ALU.max, op1=ALU.add,
                                            accum_out=accD[:, ti:ti + 1])
                # S5: layer-3 matmul + K=128 b3 inject
                ti = step - 6
                if 0 <= ti < nt:
                    z3 = psZ3.tile([H, TCOLS], F32, tag="z3", name=f"z3_{ti}")
                    nc.tensor.matmul(out=z3[:, :], lhsT=W3,
                                     rhs=a2_t.pop(ti)[:, :],
                                     start=True, stop=False)
                    nc.tensor.matmul(out=z3[:, :], lhsT=injW, rhs=oneb[:, :],
                                     start=False, stop=True)
                    z3_t[ti] = z3
                # S4: relu2 (ACT)
                ti = step - 5
                if 0 <= ti < nt:
                    a2 = abuf.tile([H, TCOLS], BF16, tag="a2", name=f"a2_{ti}")
                    nc.scalar.activation(out=a2[:, :], in_=z2_t.pop(ti)[:, :],
                                         func=AF.Relu, bias=b2, scale=1.0)
                    a2_t[ti] = a2
                # S3: layer-2 matmul
                ti = step - 4
                if 0 <= ti < nt:
                    z2 = psZ2.tile([H, TCOLS], F32, tag="z2", name=f"z2_{ti}")
                    nc.tensor.matmul(out=z2[:, :], lhsT=W2,
                                     rhs=a1_t.pop(ti)[:, :],
                                     start=True, stop=True)
                    z2_t[ti] = z2
                # S2: relu1 (b1 already in z1 via the const-1 row);
                # 8/15 ACT, 7/15 DVE to balance the engines
                ti = step - 3
                if 0 <= ti < nt:
                    a1 = abuf.tile([H, TCOLS], BF16, tag="a1", name=f"a1_{ti}")
                    z1 = z1_t.pop(ti)
                    if _r1_on_act(ti):
                        nc.scalar.activation(out=a1[:, :], in_=z1[:, :],
                                             func=AF.Relu, bias=0.0, scale=1.0)
                    else:
                        nc.vector.tensor_scalar(out=a1[:, :], in0=z1[:, :],
                                                scalar1=0.0, scalar2=0.0,
                                                op0=ALU.max, op1=ALU.add)
                    a1_t[ti] = a1
                # S1: fused layer-1 matmul (rb + type one-hots)
                ti = step - 2
                if 0 <= ti < nt:
                    z1 = psZ1.tile([H, TCOLS], F32, tag="z1", name=f"z1_{ti}")
                    nc.tensor.matmul(out=z1[:, :], lhsT=t["lhs1"][:, :],
                                     rhs=rhs_t.pop(ti)[:, :],
                                     start=True, stop=True)
                    z1_t[ti] = z1
                # S0: stream rhs tiles; 2 pre-issued for a fast start,
                # catching up to a 4-step lead (<=2 dispatches per step)
                for _ in range(2):
                    if next_rhs < min(nt, step + 5):
                        issue_rhs(next_rhs)
                        next_rhs += 1

            # ---- head ----
            tot = const.tile([H, 1], F32, tag="tot", name="tot")
            nc.vector.tensor_reduce(out=tot[:, :], in_=accD[:, :],
                                    axis=mybir.AxisListType.X, op=ALU.add)
            # repr = (tot - padc3) * recb
            repr_ = const.tile([H, 1], F32, tag="repr", name="repr_")
            nc.vector.scalar_tensor_tensor(out=repr_[:, :], in0=tot[:, :],
                                           scalar=padc3, in1=recb,
                                           op0=ALU.subtract, op1=ALU.mult)
            r1_ps = psZ2.tile([H, 1], F32, tag="z2", name="r1_ps")
            nc.tensor.matmul(out=r1_ps[:, :], lhsT=t["Wr1"][:, :],
                             rhs=repr_[:, :], start=True, stop=True)
            r1 = const.tile([H, 1], F32, tag="r1", name="r1")
            nc.scalar.activation(out=r1[:, :], in_=r1_ps[:, :], func=AF.Relu,
                                 bias=br1, scale=1.0)
            sc_ps = psZ3.tile([1, 1], F32, tag="z3", name="sc_ps")
            nc.tensor.matmul(out=sc_ps[:, :], lhsT=Wr2,
                             rhs=r1[:, :], start=True, stop=True)
            scf = const.tile([1, 1], F32, tag="scf", name="scf")
            nc.vector.scalar_tensor_tensor(out=scf[:, :], in0=sc_ps[:, :],
                                           scalar=br2, in1=gt0,
                                           op0=ALU.add, op1=ALU.mult)
            nc.sync.dma_start(out=score_ap, in_=scf[:, :])

    nc.compile()
    return nc


def _get_nc(nt):
    key = ("nc", nt)
    if key not in _CACHE:
        _CACHE[key] = _build_nc(nt)
    return _CACHE[key]


def kernel(protein_pos, ligand_pos, prot_emb, lig_emb,
           W1, b1, W2, b2, W3, b3, Wr1, br1, Wr2, br2,
           protein_atom_type, ligand_atom_type, protein_batch, ligand_batch):
    protein_pos = np.asarray(protein_pos, dtype=np.float32).reshape(B, P, 3)
    ligand_pos = np.asarray(ligand_pos, dtype=np.float32).reshape(B, L, 3)
    prot_emb = np.asarray(prot_emb, dtype=np.float32)
    lig_emb = np.asarray(lig_emb, dtype=np.float32)
    W1 = np.asarray(W1, dtype=np.float32)
    W2 = np.asarray(W2, dtype=np.float32)
    W3 = np.asarray(W3, dtype=np.float32)
    b1 = np.asarray(b1, dtype=np.float32).reshape(H)
    b2 = np.asarray(b2, dtype=np.float32).reshape(H)
    b3 = np.asarray(b3, dtype=np.float32).reshape(H)
    ptype = np.asarray(protein_atom_type).astype(np.int64).reshape(B, P)
    ltype = np.asarray(ligand_atom_type).astype(np.int64).reshape(B, L)

    # fused layer-1 weights: [W1c | prot_emb@W1a | lig_emb@W1b]
    W1a, W1b, W1c = W1[0:H], W1[H:2 * H], W1[2 * H:2 * H + RB]
    lhs1 = np.zeros((K1, H), dtype=np.float32)
    lhs1[0:RB] = W1c
    lhs1[RB:RB + NPT] = prot_emb @ W1a
    lhs1[RB + NPT:K1 - 1] = lig_emb @ W1b
    lhs1[K1 - 1] = b1
    lhs1_bf = lhs1.astype(BF)
    W2_bf = W2.astype(BF)
    W3_bf = W3.astype(BF)
    # the inject matmul computes sum_k (b3[m]/128) * 1 = b3[m]; /128 is a
    # power of two so bf16(b3/128) loses nothing over bf16(b3)
    injW = np.tile((b3 / 128.0).astype(BF)[None, :], (H, 1))
    wpack = np.concatenate([W2_bf, W3_bf, injW], axis=1)

    # padding column output, with device-matching bf16 quantization
    b1q = b1.astype(BF).astype(np.float32)
    a1p = np.maximum(b1q, 0.0).astype(BF).astype(np.float32)
    a2p = np.maximum(W2_bf.astype(np.float32).T @ a1p + b2, 0.0)
    a2p = a2p.astype(BF).astype(np.float32)
    b3q = (b3 / 128.0).astype(BF).astype(np.float32) * 128.0
    c3 = np.maximum(W3_bf.astype(np.float32).T @ a2p + b3q, 0.0)

    centers = np.linspace(0.0, CUTOFF, RB, dtype=np.float32)

    # per-complex compaction
    rb_l, pt_l, lt_l, nv_l = [], [], [], []
    for b in range(B):
        diff = protein_pos[b][:, None, :] - ligand_pos[b][None, :, :]
        dist = np.sqrt((diff * diff).sum(-1, dtype=np.float32))
        pidx, lidx = np.nonzero(dist < np.float32(CUTOFF))
        dv = dist[pidx, lidx]
        rb_l.append(np.exp(-0.5 * ((dv[:, None] - centers) / WIDTH) ** 2,
                           dtype=np.float32))
        pt_l.append(ptype[b][pidx])
        lt_l.append(ltype[b][lidx])
        nv_l.append(len(dv))

    nt = max(2, -(-max(nv_l) // TCOLS))
    cols = nt * TCOLS

    in_maps = []
    for b in range(B):
        nv = nv_l[b]
        rhs = np.zeros((K1, cols), dtype=BF)
        rhs[0:RB, :nv] = rb_l[b].T.astype(BF)
        ar = np.arange(nv)
        onehot = np.zeros((NPT + NLT, nv), dtype=BF)
        onehot[pt_l[b], ar] = 1.0
        onehot[NPT + lt_l[b], ar] = 1.0
        rhs[RB:K1 - 1, :nv] = onehot
        rhs[K1 - 1, :] = 1.0          # const-1 row: adds b1 everywhere
        # tile-contiguous layout: [nt*K1, TCOLS]
        rhs_t = np.ascontiguousarray(
            rhs.reshape(K1, nt, TCOLS).transpose(1, 0, 2)).reshape(
                nt * K1, TCOLS)
        vecs = np.zeros((H, 8), dtype=np.float32)
        vecs[:, 0] = b1
        vecs[:, 1] = b2
        vecs[:, 2] = np.asarray(br1, np.float32).reshape(H)
        vecs[:, 3] = (cols - nv) * c3
        vecs[:, 4] = 1.0 / max(nv, 1.0)
        vecs[:, 5] = np.asarray(Wr2, np.float32).reshape(H)
        vecs[0, 6] = np.float32(np.asarray(br2).reshape(()))
        vecs[0, 7] = 1.0 if nv > 0 else 0.0
        m = {
            "rhs_all": rhs_t,
            "lhs1": lhs1_bf,
            "wpack": wpack,
            "vecs": vecs,
            "Wr1": np.asarray(Wr1, np.float32),
        }
        in_maps.append(m)

    nc = _get_nc(nt)
    res = bass_utils.run_bass_kernel_spmd(nc, in_maps,
                                          core_ids=list(range(N_CORES)))
    out = np.array([res.results[b]["score"][0, 0] for b in range(B)],
                   dtype=np.float32)
    return out
